# revision 39
# baseline (speedup 1.0000x reference)
"""Trainium2 Bass kernel for AccumulativeGainLoss.

Data-parallel over B across 8 NeuronCores (2 batch elements per core).

Math restructuring (validated to ~2.5e-6 rel err in f32 vs the jax reference):
for each batch element b, with F = preds[b] [N,K] and Y = y_ts[b] rearranged
to [N, T*D]:
    H   = [F|1]^T [F|1]                 (Gram + column sums + N)
    inv = (F^T F)^{-1}                  (Newton-Schulz, 5 iters, X0=(K/tr)I)
    M   = F^T Y, sumy = 1^T Y, sy2 = 1^T (Y*Y)
    q   = colsum(M * (inv M))           (= diag(M^T inv M))
    ss_res = sy2 - q                    (beta^T FtF beta ≈ beta^T M, err ~1e-12)
    ss_tot = sy2 - sumy^2/N + EPS
    r2  = 1 - ss_res/ss_tot ;  wsum_b = sum_td w[t,d] * r2[t,d]
    cov = FtF - s s^T / N ; c = 1/diag(cov) ; quad_b = c^T (cov*cov) c
loss = mean_b( -wsum_b/T ) + 0.1 * mean_b( quad_b - K )

The big tensor Y (12.3 MB/core) is streamed through SBUF once; all
reductions over N happen on the TensorEngine (PSUM accumulation over
47 chunks of 128 rows). sy2 needs Y^2, computed on ScalarE.
"""

import ml_dtypes
import numpy as np

import concourse.bacc as bacc
import concourse.bass as bass
import concourse.mybir as mybir
import concourse.tile as tile
from concourse.bass_utils import run_bass_kernel_spmd
from concourse.tile_rust import add_dep_helper

F32 = mybir.dt.float32
BF16 = mybir.dt.bfloat16
ALU = mybir.AluOpType
AX = mybir.AxisListType

B, T, N, K, D = 16, 32, 6000, 32, 8
NCORES = 8
JB = B // NCORES          # batch elements per core
NCH = 47                  # ceil(6000/128) chunks of 128 rows
NPAD = NCH * 128          # 6016
TD = T * D                # 256
FW = 34                   # per-chunk F block: 32 coeffs + ones col + pad
FROW = NCH * FW           # 1598
YROW = NCH * TD           # 12032
BLOCKS = (6, 6, 6, 6, 6, 6, 6, 5)  # chunk blocking of the Y stream
NS_ITERS = 4
EPS = 1e-8
DECAY = 0.9
PEN = 0.1

_CACHE = {}


def _build_program():
    nc = bacc.Bacc("TRN2", target_bir_lowering=False, debug=False)
    y_d = nc.declare_dram_parameter("y", [JB, 128, YROW], BF16, isOutput=False)
    f_d = nc.declare_dram_parameter("f", [JB, 128, FROW], BF16, isOutput=False)
    c_d = nc.declare_dram_parameter("c32", [32, 96], F32, isOutput=False)
    w_d = nc.declare_dram_parameter("w2", [1, TD], F32, isOutput=False)
    o_d = nc.declare_dram_parameter("out", [1, 2], F32, isOutput=True)

    with tile.TileContext(nc) as tc:
        with (
            tc.tile_pool(name="cpool", bufs=1) as cpool,
            tc.tile_pool(name="fpool", bufs=1) as fpool,
            tc.tile_pool(name="ypool", bufs=8) as ypool,
            tc.tile_pool(name="nsb", bufs=2) as nsb,
            tc.tile_pool(name="esb", bufs=2) as esb,
            # PSUM is 8 banks; every tag below occupies one bank.
            tc.tile_pool(name="ps", bufs=1, space="PSUM") as ps,
        ):
            # ---- PE warmup: junk matmuls fill the otherwise idle start
            # window so the HAM clock-gate reaches 2.4 GHz before the real
            # matmuls arrive (~3.4 us of sustained activity required).
            wtile = cpool.tile([128, 512], BF16)
            nc.gpsimd.memset(wtile, 0.01)
            wps = ps.tile([128, 512], F32, tag="wrm")
            for _ in range(18):
                nc.tensor.matmul(wps, wtile[:, 0:128], wtile,
                                 start=True, stop=True)

            # ---- DMAs: F first (needed by every matmul), then the Y
            # stream; triggers alternate between the two HWDGE issuing
            # engines (SP / ACT sequencer) so trigger issue is not serial
            # on one queue.
            ftile = fpool.tile([128, JB * FROW], BF16)
            fdma = nc.sync.dma_start(
                out=ftile[:, :].rearrange("p (j r) -> p j r", j=JB),
                in_=f_d[:, :, :].rearrange("j p r -> p j r"),
            )

            def fch(j, c):  # chunk-c F block [128, 33] (coeffs + ones)
                return ftile[:, j * FROW + c * FW: j * FROW + c * FW + 33]

            # ycomb tiles: [Y | Ysq] halves, one tile per (j, block).
            # The transfers are chained depth-2 (each trigger waits for the
            # completion two links back): at most two Y streams in flight,
            # so blocks arrive in order every ~2 us at full aggregate HBM
            # bandwidth instead of all 16 landing together at the end
            # (SDMA engines round-robin between all queues that have work).
            ycombs = {}
            ydmas = []
            dma_engines = [nc.sync, nc.scalar]
            for j in range(JB):
                c0 = 0
                for bi, blk in enumerate(BLOCKS):
                    yc = ypool.tile([128, blk * 512], BF16, tag="yc")
                    eng = dma_engines[(j * len(BLOCKS) + bi) % 2]
                    dma = eng.dma_start(
                        out=yc[:, 0:blk * TD],
                        in_=y_d[j, :, c0 * TD:(c0 + blk) * TD],
                    )
                    k = len(ydmas)
                    if k == 0:
                        pass  # block 0 streams alongside F
                    elif k < 4:
                        add_dep_helper(dma.ins, fdma.ins, sync=True,
                                       reason="Y ramp waits for F")
                    else:
                        add_dep_helper(dma.ins, ydmas[k - 3].ins, sync=True,
                                       reason="depth-3 Y stream chain")
                    ydmas.append(dma)
                    ycombs[(j, bi)] = yc
                    c0 += blk

            consts = cpool.tile([32, 96], F32)
            nc.gpsimd.dma_start(out=consts, in_=c_d[:, :])
            eye = consts[:, 0:32]
            twoI = consts[:, 32:64]
            ones2d = consts[:, 64:96]
            ones32 = consts[:, 64:65]

            w2sb = cpool.tile([33, TD], F32)
            nc.gpsimd.dma_start(out=w2sb[32:33, :], in_=w_d[:, :])

            # ---- Newton-Schulz inverse of FtF + correlation penalty, per j.
            # The Gram matrices are computed up front (dense PE work), but
            # the serial NS/corr chains (tiny matmul <-> DVE ping-pong)
            # would head-of-line-block the PE FIFO if emitted as one run.
            # Each PE step is wrapped in a closure and interleaved into the
            # streaming chunk loop below, so every step's DVE inputs are
            # long finished before the PE reaches its matmul.
            inv_sb = [None, None]
            quad_sb = [None, None]
            Hsb_j = [None, None]

            def emit_H(j):
                Hps = ps.tile([33, 33], F32, tag=f"H{j}")
                for c in range(NCH):
                    nc.tensor.matmul(
                        Hps, fch(j, c), fch(j, c),
                        start=(c == 0), stop=(c == NCH - 1),
                    )
                Hsb = nsb.tile([33, 33], F32, tag="Hsb")
                nc.vector.tensor_copy(Hsb, Hps)
                Hsb_j[j] = Hsb

            def make_steps(j):
                state = {}

                def s_trace():
                    Hsb = Hsb_j[j]
                    A = state["A"] = Hsb[0:32, 0:32]
                    state["s_row"] = Hsb[32:33, 0:32]
                    dm = nsb.tile([32, 32], F32, tag="dm")
                    nc.vector.tensor_mul(dm, A, eye)
                    dg = nsb.tile([32, 1], F32, tag="dg")
                    nc.vector.reduce_sum(dg, dm, axis=AX.X)
                    trp = ps.tile([32, 32], F32, tag="tns", bufs=2)
                    nc.tensor.matmul(trp[:, 0:1], ones2d, dg,
                                     start=True, stop=True)
                    rtr = nsb.tile([32, 1], F32, tag="rtr")
                    nc.vector.reciprocal(rtr, trp[:, 0:1])
                    c0v = nsb.tile([32, 1], F32, tag="c0v")
                    nc.vector.tensor_scalar_mul(c0v, rtr, float(K))
                    X = nsb.tile([32, 32], F32, tag="Xns", bufs=2 * NS_ITERS + 4)
                    nc.vector.tensor_scalar(X, eye, c0v, None, ALU.mult)
                    state["X"] = X
                steps = [s_trace]

                def ns_a():
                    t1 = ps.tile([32, 32], F32, tag="tns", bufs=2)
                    nc.tensor.matmul(t1, state["A"], state["X"],
                                     start=True, stop=True)
                    z = nsb.tile([32, 32], F32, tag="Zns",
                                 bufs=2 * NS_ITERS + 2)
                    nc.vector.tensor_sub(z, twoI, t1)
                    state["z"] = z

                def ns_b():
                    x2 = ps.tile([32, 32], F32, tag="tns", bufs=2)
                    nc.tensor.matmul(x2, state["X"], state["z"],
                                     start=True, stop=True)
                    Xn = nsb.tile([32, 32], F32, tag="Xns",
                                  bufs=2 * NS_ITERS + 4)
                    nc.vector.tensor_copy(Xn, x2)
                    state["X"] = Xn
                for _ in range(NS_ITERS):
                    steps += [ns_a, ns_b]

                def c_outer():
                    inv_sb[j] = state["X"]
                    outp = ps.tile([32, 32], F32, tag="tns", bufs=2)
                    nc.tensor.matmul(outp, state["s_row"], state["s_row"],
                                     start=True, stop=True)
                    covn = nsb.tile([32, 32], F32, tag="covn")
                    nc.vector.tensor_scalar_mul(covn, outp, 1.0 / N)
                    cov = nsb.tile([32, 32], F32, tag="cov")
                    nc.vector.tensor_sub(cov, state["A"], covn)
                    dm2 = nsb.tile([32, 32], F32, tag="dm2")
                    nc.vector.tensor_mul(dm2, cov, eye)
                    dg2 = nsb.tile([32, 1], F32, tag="dg2")
                    nc.vector.reduce_sum(dg2, dm2, axis=AX.X)
                    cv = nsb.tile([32, 1], F32, tag="cv")
                    nc.vector.reciprocal(cv, dg2)
                    A2 = nsb.tile([32, 32], F32, tag="A2")
                    nc.vector.tensor_mul(A2, cov, cov)
                    state["cv"] = cv
                    state["A2"] = A2

                def c_u():
                    ups = ps.tile([32, 32], F32, tag="tns", bufs=2)
                    nc.tensor.matmul(ups[:, 0:1], state["A2"], state["cv"],
                                     start=True, stop=True)
                    usb = nsb.tile([32, 1], F32, tag="usb")
                    nc.vector.tensor_copy(usb, ups[:, 0:1])
                    state["usb"] = usb

                def c_q():
                    qd = ps.tile([33, 32], F32, tag="tns", bufs=2)
                    nc.tensor.matmul(qd[32:33, 0:1], state["usb"], state["cv"],
                                     start=True, stop=True)
                    qsb = nsb.tile([33, 1], F32, tag="qsb")
                    nc.vector.tensor_copy(qsb[32:33, :], qd[32:33, 0:1])
                    quad_sb[j] = qsb
                steps += [c_outer, c_u, c_q]
                return steps

            pending = {0: make_steps(0), 1: make_steps(1)}

            # results staging: [wsum0, wsum1, quad0, quad1] (on partition 32,
            # where the GS row outputs live)
            wsout = cpool.tile([33, 4], F32)

            # ---- stream: square each block (alternating ScalarE / DVE),
            # then one matmul per chunk with rhs spanning [Y | Ysq]:
            #   GS[0:32, 0:256]   = F^T Y   (M)
            #   GS[32,   0:256]   = 1^T Y   (sumy)
            #   GS[32,   256:512] = 1^T Y^2 (sy2)
            for j in range(JB):
                GS = ps.tile([33, 512], F32, tag=f"GS{j}")
                steps = pending.pop(j)
                c0 = 0
                for bi, blk in enumerate(BLOCKS):
                    yc = ycombs[(j, bi)]
                    # squares run on ScalarE only: DVE stays responsive for
                    # the interleaved Newton-Schulz / epilogue chains
                    for p0 in range(0, blk, 6):
                        p1 = min(p0 + 6, blk)
                        nc.scalar.square(
                            yc[:, (blk + p0) * TD:(blk + p1) * TD],
                            yc[:, p0 * TD:p1 * TD],
                        )
                    rhs2 = yc[:, :].rearrange("p (two cd) -> p two cd", two=2)
                    for lc in range(blk):
                        c = c0 + lc
                        nc.tensor.matmul(
                            GS, fch(j, c),
                            rhs2[:, :, lc * TD:(lc + 1) * TD],
                            start=(c == 0), stop=(c == NCH - 1),
                        )
                        if (j > 0 or c >= 12) and c % 4 == 3 and steps:
                            steps.pop(0)()
                    if j == 0 and bi == 0:
                        emit_H(0)
                    if j == 0 and bi == 1:
                        emit_H(1)
                    c0 += blk
                while steps:
                    steps.pop(0)()

                # ---- per-j epilogue
                Gsb = esb.tile([33, 512], F32, tag="Gsb")
                nc.vector.tensor_copy(Gsb, GS)
                M = Gsb[0:32, 0:TD]
                sumy = Gsb[32:33, 0:TD]
                sy2row = Gsb[32:33, TD:2 * TD]

                Pps = ps.tile([32, TD], F32, tag="tPq")
                nc.tensor.matmul(Pps, inv_sb[j], M, start=True, stop=True)
                # ss_tot chain runs on DVE while PE computes P = inv M
                sumy2 = esb.tile([33, TD], F32, tag="sumy2")
                nc.vector.tensor_mul(sumy2[32:33, :], sumy, sumy)
                sstot_a = esb.tile([33, TD], F32, tag="sstot_a")
                nc.vector.tensor_scalar(
                    sstot_a[32:33, :], sumy2[32:33, :], -1.0 / N, EPS,
                    ALU.mult, ALU.add
                )
                sstot = esb.tile([33, TD], F32, tag="sstot")
                nc.vector.tensor_add(sstot[32:33, :], sstot_a[32:33, :], sy2row)
                rec = esb.tile([33, TD], F32, tag="rec")
                nc.vector.reciprocal(rec[32:33, :], sstot[32:33, :])
                W = esb.tile([32, TD], F32, tag="W")
                nc.vector.tensor_mul(W, M, Pps)
                qps = ps.tile([33, TD], F32, tag="tPq")
                nc.tensor.matmul(qps[32:33, :], ones32, W, start=True, stop=True)
                ssres = esb.tile([33, TD], F32, tag="ssres")
                nc.vector.tensor_sub(ssres[32:33, :], sy2row, qps[32:33, :])
                ratio = esb.tile([33, TD], F32, tag="ratio")
                nc.vector.tensor_mul(ratio[32:33, :], ssres[32:33, :],
                                     rec[32:33, :])
                r2 = esb.tile([33, TD], F32, tag="r2")
                nc.vector.tensor_scalar(r2[32:33, :], ratio[32:33, :],
                                        -1.0, 1.0, ALU.mult, ALU.add)
                scratch = esb.tile([33, TD], F32, tag="scratch")
                nc.vector.tensor_mul(scratch[32:33, :], r2[32:33, :],
                                     w2sb[32:33, :])
                nc.vector.reduce_sum(wsout[32:33, j:j + 1], scratch[32:33, :],
                                     axis=AX.X)
                nc.vector.tensor_copy(wsout[32:33, 2 + j:3 + j],
                                      quad_sb[j][32:33, :])

            outsb = cpool.tile([33, 2], F32)
            nc.vector.tensor_add(outsb[32:33, 0:1], wsout[32:33, 0:1],
                                 wsout[32:33, 1:2])
            nc.vector.tensor_add(outsb[32:33, 1:2], wsout[32:33, 2:3],
                                 wsout[32:33, 3:4])
            nc.gpsimd.dma_start(out=o_d[:, :], in_=outsb[32:33, :])

    nc.compile()
    return nc


def _prepare_in_maps(preds, y_ts, importance):
    preds = np.ascontiguousarray(preds, dtype=np.float32)
    y_ts = np.ascontiguousarray(y_ts, dtype=np.float32)
    importance = np.ascontiguousarray(importance, dtype=np.float32)

    bf16 = ml_dtypes.bfloat16

    # Y image: yimg[b, p, c*TD + t*D + d] = y_ts[b, t, c*128+p, d]
    ypad = np.zeros((B, T, NPAD, D), dtype=bf16)
    ypad[:, :, :N, :] = y_ts.astype(bf16)
    yimg = np.ascontiguousarray(
        ypad.reshape(B, T, NCH, 128, D).transpose(0, 3, 2, 1, 4)
    ).reshape(B, 128, YROW)

    # F image: fimg[b, p, c*FW + k] = preds[b, c*128+p, k]; col 32 = valid-mask
    fpad = np.zeros((B, NPAD, FW), dtype=bf16)
    fpad[:, :N, :K] = preds.astype(bf16)
    fpad[:, :N, K] = 1.0
    fimg = np.ascontiguousarray(
        fpad.reshape(B, NCH, 128, FW).transpose(0, 2, 1, 3)
    ).reshape(B, 128, FROW)

    c32 = np.zeros((32, 96), dtype=np.float32)
    c32[:, 0:32] = np.eye(32, dtype=np.float32)
    c32[:, 32:64] = 2.0 * np.eye(32, dtype=np.float32)
    c32[:, 64:96] = 1.0

    decay = DECAY ** np.arange(T, dtype=np.float32)
    w2 = (decay[:, None] * importance[None, :].astype(np.float32)).reshape(1, TD)
    w2 = np.ascontiguousarray(w2, dtype=np.float32)

    in_maps = []
    for i in range(NCORES):
        in_maps.append({
            "y": np.ascontiguousarray(yimg[i * JB:(i + 1) * JB]),
            "f": np.ascontiguousarray(fimg[i * JB:(i + 1) * JB]),
            "c32": c32,
            "w2": w2,
        })
    return in_maps


def _combine(results):
    loss = 0.0
    for r in results:
        w_total, q_total = float(r["out"][0, 0]), float(r["out"][0, 1])
        loss += (-w_total / T + PEN * (q_total - JB * K)) / B
    return np.float32(loss)


def run_on_device(preds, y_ts, importance, trace=False, **spmd_kwargs):
    if "nc" not in _CACHE:
        _CACHE["nc"] = _build_program()
    nc = _CACHE["nc"]
    in_maps = _prepare_in_maps(preds, y_ts, importance)
    res = run_bass_kernel_spmd(
        nc, in_maps, list(range(NCORES)), trace=trace, **spmd_kwargs
    )
    return _combine(res.results), res


def kernel(preds, y_ts, importance):
    loss, _ = run_on_device(preds, y_ts, importance, trace=False)
    return loss


# revision 40
# speedup vs baseline: 1.0241x; 1.0241x over previous
"""Trainium2 Bass kernel for AccumulativeGainLoss.

Data-parallel over B across 8 NeuronCores (2 batch elements per core).

Math restructuring (validated to ~2.5e-6 rel err in f32 vs the jax reference):
for each batch element b, with F = preds[b] [N,K] and Y = y_ts[b] rearranged
to [N, T*D]:
    H   = [F|1]^T [F|1]                 (Gram + column sums + N)
    inv = (F^T F)^{-1}                  (Newton-Schulz, 5 iters, X0=(K/tr)I)
    M   = F^T Y, sumy = 1^T Y, sy2 = 1^T (Y*Y)
    q   = colsum(M * (inv M))           (= diag(M^T inv M))
    ss_res = sy2 - q                    (beta^T FtF beta ≈ beta^T M, err ~1e-12)
    ss_tot = sy2 - sumy^2/N + EPS
    r2  = 1 - ss_res/ss_tot ;  wsum_b = sum_td w[t,d] * r2[t,d]
    cov = FtF - s s^T / N ; c = 1/diag(cov) ; quad_b = c^T (cov*cov) c
loss = mean_b( -wsum_b/T ) + 0.1 * mean_b( quad_b - K )

The big tensor Y (12.3 MB/core) is streamed through SBUF once; all
reductions over N happen on the TensorEngine (PSUM accumulation over
47 chunks of 128 rows). sy2 needs Y^2, computed on ScalarE.
"""

import ml_dtypes
import numpy as np

import concourse.bacc as bacc
import concourse.bass as bass
import concourse.mybir as mybir
import concourse.tile as tile
from concourse.bass_utils import run_bass_kernel_spmd
from concourse.tile_rust import add_dep_helper

F32 = mybir.dt.float32
BF16 = mybir.dt.bfloat16
ALU = mybir.AluOpType
AX = mybir.AxisListType

B, T, N, K, D = 16, 32, 6000, 32, 8
NCORES = 8
JB = B // NCORES          # batch elements per core
NCH = 47                  # ceil(6000/128) chunks of 128 rows
NPAD = NCH * 128          # 6016
TD = T * D                # 256
FW = 34                   # per-chunk F block: 32 coeffs + ones col + pad
FROW = NCH * FW           # 1598
YROW = NCH * TD           # 12032
BLOCKS = (6, 6, 6, 6, 6, 6, 6, 5)  # chunk blocking of the Y stream
NS_ITERS = 4
EPS = 1e-8
DECAY = 0.9
PEN = 0.1

_CACHE = {}


def _build_program():
    nc = bacc.Bacc("TRN2", target_bir_lowering=False, debug=False)
    y_d = nc.declare_dram_parameter("y", [JB, 128, YROW], BF16, isOutput=False)
    f_d = nc.declare_dram_parameter("f", [JB, 128, FROW], BF16, isOutput=False)
    c_d = nc.declare_dram_parameter("c32", [32, 96], F32, isOutput=False)
    w_d = nc.declare_dram_parameter("w2", [1, TD], F32, isOutput=False)
    o_d = nc.declare_dram_parameter("out", [1, 2], F32, isOutput=True)

    with tile.TileContext(nc) as tc:
        with (
            tc.tile_pool(name="cpool", bufs=1) as cpool,
            tc.tile_pool(name="fpool", bufs=1) as fpool,
            tc.tile_pool(name="ypool", bufs=8) as ypool,
            tc.tile_pool(name="nsb", bufs=2) as nsb,
            tc.tile_pool(name="esb", bufs=2) as esb,
            # PSUM is 8 banks; every tag below occupies one bank.
            tc.tile_pool(name="ps", bufs=1, space="PSUM") as ps,
        ):
            # ---- PE warmup: junk matmuls fill the otherwise idle start
            # window so the HAM clock-gate reaches 2.4 GHz before the real
            # matmuls arrive (~3.4 us of sustained activity required).
            wtile = cpool.tile([128, 512], BF16)
            nc.gpsimd.memset(wtile, 0.01)
            wps = ps.tile([128, 512], F32, tag="wrm")
            for _ in range(14):
                nc.tensor.matmul(wps, wtile[:, 0:128], wtile,
                                 start=True, stop=True)

            # ---- DMAs: F first (needed by every matmul), then the Y
            # stream; triggers alternate between the two HWDGE issuing
            # engines (SP / ACT sequencer) so trigger issue is not serial
            # on one queue.
            ftile = fpool.tile([128, JB * FROW], BF16)
            fdma = nc.sync.dma_start(
                out=ftile[:, :].rearrange("p (j r) -> p j r", j=JB),
                in_=f_d[:, :, :].rearrange("j p r -> p j r"),
            )

            def fch(j, c):  # chunk-c F block [128, 33] (coeffs + ones)
                return ftile[:, j * FROW + c * FW: j * FROW + c * FW + 33]

            # ycomb tiles: [Y | Ysq] halves, one tile per (j, block).
            # The transfers are chained depth-2 (each trigger waits for the
            # completion two links back): at most two Y streams in flight,
            # so blocks arrive in order every ~2 us at full aggregate HBM
            # bandwidth instead of all 16 landing together at the end
            # (SDMA engines round-robin between all queues that have work).
            ycombs = {}
            ydmas = []
            dma_engines = [nc.sync, nc.scalar]
            for j in range(JB):
                c0 = 0
                for bi, blk in enumerate(BLOCKS):
                    yc = ypool.tile([128, blk * 512], BF16, tag="yc")
                    eng = dma_engines[(j * len(BLOCKS) + bi) % 2]
                    dma = eng.dma_start(
                        out=yc[:, 0:blk * TD],
                        in_=y_d[j, :, c0 * TD:(c0 + blk) * TD],
                    )
                    k = len(ydmas)
                    if k < 3:
                        add_dep_helper(dma.ins, fdma.ins, sync=True,
                                       reason="F streams solo first")
                    else:
                        add_dep_helper(dma.ins, ydmas[k - 3].ins, sync=True,
                                       reason="depth-3 Y stream chain")
                    ydmas.append(dma)
                    ycombs[(j, bi)] = yc
                    c0 += blk

            consts = cpool.tile([32, 96], F32)
            nc.gpsimd.dma_start(out=consts, in_=c_d[:, :])
            eye = consts[:, 0:32]
            twoI = consts[:, 32:64]
            ones2d = consts[:, 64:96]
            ones32 = consts[:, 64:65]

            w2sb = cpool.tile([33, TD], F32)
            nc.gpsimd.dma_start(out=w2sb[32:33, :], in_=w_d[:, :])

            # ---- Newton-Schulz inverse of FtF + correlation penalty, per j.
            # The Gram matrices are computed up front (dense PE work), but
            # the serial NS/corr chains (tiny matmul <-> DVE ping-pong)
            # would head-of-line-block the PE FIFO if emitted as one run.
            # Each PE step is wrapped in a closure and interleaved into the
            # streaming chunk loop below, so every step's DVE inputs are
            # long finished before the PE reaches its matmul.
            inv_sb = [None, None]
            quad_sb = [None, None]
            Hsb_j = [None, None]

            def emit_H(j):
                Hps = ps.tile([33, 33], F32, tag=f"H{j}")
                for c in range(NCH):
                    nc.tensor.matmul(
                        Hps, fch(j, c), fch(j, c),
                        start=(c == 0), stop=(c == NCH - 1),
                    )
                Hsb = nsb.tile([33, 33], F32, tag="Hsb")
                nc.vector.tensor_copy(Hsb, Hps)
                Hsb_j[j] = Hsb

            def make_steps(j):
                state = {}

                def s_trace():
                    Hsb = Hsb_j[j]
                    A = state["A"] = Hsb[0:32, 0:32]
                    state["s_row"] = Hsb[32:33, 0:32]
                    dm = nsb.tile([32, 32], F32, tag="dm")
                    nc.vector.tensor_mul(dm, A, eye)
                    dg = nsb.tile([32, 1], F32, tag="dg")
                    nc.vector.reduce_sum(dg, dm, axis=AX.X)
                    trp = ps.tile([32, 32], F32, tag="tns", bufs=2)
                    nc.tensor.matmul(trp[:, 0:1], ones2d, dg,
                                     start=True, stop=True)
                    rtr = nsb.tile([32, 1], F32, tag="rtr")
                    nc.vector.reciprocal(rtr, trp[:, 0:1])
                    c0v = nsb.tile([32, 1], F32, tag="c0v")
                    nc.vector.tensor_scalar_mul(c0v, rtr, float(K))
                    X = nsb.tile([32, 32], F32, tag="Xns", bufs=2 * NS_ITERS + 4)
                    nc.vector.tensor_scalar(X, eye, c0v, None, ALU.mult)
                    state["X"] = X
                steps = [s_trace]

                def ns_a():
                    t1 = ps.tile([32, 32], F32, tag="tns", bufs=2)
                    nc.tensor.matmul(t1, state["A"], state["X"],
                                     start=True, stop=True)
                    z = nsb.tile([32, 32], F32, tag="Zns",
                                 bufs=2 * NS_ITERS + 2)
                    nc.vector.tensor_sub(z, twoI, t1)
                    state["z"] = z

                def ns_b():
                    x2 = ps.tile([32, 32], F32, tag="tns", bufs=2)
                    nc.tensor.matmul(x2, state["X"], state["z"],
                                     start=True, stop=True)
                    Xn = nsb.tile([32, 32], F32, tag="Xns",
                                  bufs=2 * NS_ITERS + 4)
                    nc.vector.tensor_copy(Xn, x2)
                    state["X"] = Xn
                for _ in range(NS_ITERS):
                    steps += [ns_a, ns_b]

                def c_outer():
                    inv_sb[j] = state["X"]
                    outp = ps.tile([32, 32], F32, tag="tns", bufs=2)
                    nc.tensor.matmul(outp, state["s_row"], state["s_row"],
                                     start=True, stop=True)
                    covn = nsb.tile([32, 32], F32, tag="covn")
                    nc.vector.tensor_scalar_mul(covn, outp, 1.0 / N)
                    cov = nsb.tile([32, 32], F32, tag="cov")
                    nc.vector.tensor_sub(cov, state["A"], covn)
                    dm2 = nsb.tile([32, 32], F32, tag="dm2")
                    nc.vector.tensor_mul(dm2, cov, eye)
                    dg2 = nsb.tile([32, 1], F32, tag="dg2")
                    nc.vector.reduce_sum(dg2, dm2, axis=AX.X)
                    cv = nsb.tile([32, 1], F32, tag="cv")
                    nc.vector.reciprocal(cv, dg2)
                    A2 = nsb.tile([32, 32], F32, tag="A2")
                    nc.vector.tensor_mul(A2, cov, cov)
                    state["cv"] = cv
                    state["A2"] = A2

                def c_u():
                    ups = ps.tile([32, 32], F32, tag="tns", bufs=2)
                    nc.tensor.matmul(ups[:, 0:1], state["A2"], state["cv"],
                                     start=True, stop=True)
                    usb = nsb.tile([32, 1], F32, tag="usb")
                    nc.vector.tensor_copy(usb, ups[:, 0:1])
                    state["usb"] = usb

                def c_q():
                    qd = ps.tile([33, 32], F32, tag="tns", bufs=2)
                    nc.tensor.matmul(qd[32:33, 0:1], state["usb"], state["cv"],
                                     start=True, stop=True)
                    qsb = nsb.tile([33, 1], F32, tag="qsb")
                    nc.vector.tensor_copy(qsb[32:33, :], qd[32:33, 0:1])
                    quad_sb[j] = qsb
                steps += [c_outer, c_u, c_q]
                return steps

            emit_H(0)
            emit_H(1)
            pending = {0: make_steps(0), 1: make_steps(1)}

            # results staging: [wsum0, wsum1, quad0, quad1] (on partition 32,
            # where the GS row outputs live)
            wsout = cpool.tile([33, 4], F32)

            # ---- stream: square each block (alternating ScalarE / DVE),
            # then one matmul per chunk with rhs spanning [Y | Ysq]:
            #   GS[0:32, 0:256]   = F^T Y   (M)
            #   GS[32,   0:256]   = 1^T Y   (sumy)
            #   GS[32,   256:512] = 1^T Y^2 (sy2)
            for j in range(JB):
                GS = ps.tile([33, 512], F32, tag=f"GS{j}")
                steps = pending.pop(j)
                c0 = 0
                for bi, blk in enumerate(BLOCKS):
                    yc = ycombs[(j, bi)]
                    # squares run on ScalarE only: DVE stays responsive for
                    # the interleaved Newton-Schulz / epilogue chains
                    for p0 in range(0, blk, 6):
                        p1 = min(p0 + 6, blk)
                        nc.scalar.square(
                            yc[:, (blk + p0) * TD:(blk + p1) * TD],
                            yc[:, p0 * TD:p1 * TD],
                        )
                    rhs2 = yc[:, :].rearrange("p (two cd) -> p two cd", two=2)
                    for lc in range(blk):
                        c = c0 + lc
                        nc.tensor.matmul(
                            GS, fch(j, c),
                            rhs2[:, :, lc * TD:(lc + 1) * TD],
                            start=(c == 0), stop=(c == NCH - 1),
                        )
                        if c % 4 == 3 and steps:
                            steps.pop(0)()
                    c0 += blk
                while steps:
                    steps.pop(0)()

                # ---- per-j epilogue
                Gsb = esb.tile([33, 512], F32, tag="Gsb")
                nc.vector.tensor_copy(Gsb, GS)
                M = Gsb[0:32, 0:TD]
                sumy = Gsb[32:33, 0:TD]
                sy2row = Gsb[32:33, TD:2 * TD]

                Pps = ps.tile([32, TD], F32, tag="tPq")
                nc.tensor.matmul(Pps, inv_sb[j], M, start=True, stop=True)
                # ss_tot chain runs on DVE while PE computes P = inv M
                sumy2 = esb.tile([33, TD], F32, tag="sumy2")
                nc.vector.tensor_mul(sumy2[32:33, :], sumy, sumy)
                sstot_a = esb.tile([33, TD], F32, tag="sstot_a")
                nc.vector.tensor_scalar(
                    sstot_a[32:33, :], sumy2[32:33, :], -1.0 / N, EPS,
                    ALU.mult, ALU.add
                )
                sstot = esb.tile([33, TD], F32, tag="sstot")
                nc.vector.tensor_add(sstot[32:33, :], sstot_a[32:33, :], sy2row)
                rec = esb.tile([33, TD], F32, tag="rec")
                nc.vector.reciprocal(rec[32:33, :], sstot[32:33, :])
                W = esb.tile([32, TD], F32, tag="W")
                nc.vector.tensor_mul(W, M, Pps)
                qps = ps.tile([33, TD], F32, tag="tPq")
                nc.tensor.matmul(qps[32:33, :], ones32, W, start=True, stop=True)
                ssres = esb.tile([33, TD], F32, tag="ssres")
                nc.vector.tensor_sub(ssres[32:33, :], sy2row, qps[32:33, :])
                ratio = esb.tile([33, TD], F32, tag="ratio")
                nc.vector.tensor_mul(ratio[32:33, :], ssres[32:33, :],
                                     rec[32:33, :])
                r2 = esb.tile([33, TD], F32, tag="r2")
                nc.vector.tensor_scalar(r2[32:33, :], ratio[32:33, :],
                                        -1.0, 1.0, ALU.mult, ALU.add)
                scratch = esb.tile([33, TD], F32, tag="scratch")
                nc.vector.tensor_mul(scratch[32:33, :], r2[32:33, :],
                                     w2sb[32:33, :])
                nc.vector.reduce_sum(wsout[32:33, j:j + 1], scratch[32:33, :],
                                     axis=AX.X)
                nc.vector.tensor_copy(wsout[32:33, 2 + j:3 + j],
                                      quad_sb[j][32:33, :])

            outsb = cpool.tile([33, 2], F32)
            nc.vector.tensor_add(outsb[32:33, 0:1], wsout[32:33, 0:1],
                                 wsout[32:33, 1:2])
            nc.vector.tensor_add(outsb[32:33, 1:2], wsout[32:33, 2:3],
                                 wsout[32:33, 3:4])
            nc.gpsimd.dma_start(out=o_d[:, :], in_=outsb[32:33, :])

    nc.compile()
    return nc


def _prepare_in_maps(preds, y_ts, importance):
    preds = np.ascontiguousarray(preds, dtype=np.float32)
    y_ts = np.ascontiguousarray(y_ts, dtype=np.float32)
    importance = np.ascontiguousarray(importance, dtype=np.float32)

    bf16 = ml_dtypes.bfloat16

    # Y image: yimg[b, p, c*TD + t*D + d] = y_ts[b, t, c*128+p, d]
    ypad = np.zeros((B, T, NPAD, D), dtype=bf16)
    ypad[:, :, :N, :] = y_ts.astype(bf16)
    yimg = np.ascontiguousarray(
        ypad.reshape(B, T, NCH, 128, D).transpose(0, 3, 2, 1, 4)
    ).reshape(B, 128, YROW)

    # F image: fimg[b, p, c*FW + k] = preds[b, c*128+p, k]; col 32 = valid-mask
    fpad = np.zeros((B, NPAD, FW), dtype=bf16)
    fpad[:, :N, :K] = preds.astype(bf16)
    fpad[:, :N, K] = 1.0
    fimg = np.ascontiguousarray(
        fpad.reshape(B, NCH, 128, FW).transpose(0, 2, 1, 3)
    ).reshape(B, 128, FROW)

    c32 = np.zeros((32, 96), dtype=np.float32)
    c32[:, 0:32] = np.eye(32, dtype=np.float32)
    c32[:, 32:64] = 2.0 * np.eye(32, dtype=np.float32)
    c32[:, 64:96] = 1.0

    decay = DECAY ** np.arange(T, dtype=np.float32)
    w2 = (decay[:, None] * importance[None, :].astype(np.float32)).reshape(1, TD)
    w2 = np.ascontiguousarray(w2, dtype=np.float32)

    in_maps = []
    for i in range(NCORES):
        in_maps.append({
            "y": np.ascontiguousarray(yimg[i * JB:(i + 1) * JB]),
            "f": np.ascontiguousarray(fimg[i * JB:(i + 1) * JB]),
            "c32": c32,
            "w2": w2,
        })
    return in_maps


def _combine(results):
    loss = 0.0
    for r in results:
        w_total, q_total = float(r["out"][0, 0]), float(r["out"][0, 1])
        loss += (-w_total / T + PEN * (q_total - JB * K)) / B
    return np.float32(loss)


def run_on_device(preds, y_ts, importance, trace=False, **spmd_kwargs):
    if "nc" not in _CACHE:
        _CACHE["nc"] = _build_program()
    nc = _CACHE["nc"]
    in_maps = _prepare_in_maps(preds, y_ts, importance)
    res = run_bass_kernel_spmd(
        nc, in_maps, list(range(NCORES)), trace=trace, **spmd_kwargs
    )
    return _combine(res.results), res


def kernel(preds, y_ts, importance):
    loss, _ = run_on_device(preds, y_ts, importance, trace=False)
    return loss


# revision 41
# speedup vs baseline: 1.1918x; 1.1638x over previous
"""Trainium2 Bass kernel for AccumulativeGainLoss.

Data-parallel over B across 8 NeuronCores (2 batch elements per core).

Math restructuring (validated to ~2.5e-6 rel err in f32 vs the jax reference):
for each batch element b, with F = preds[b] [N,K] and Y = y_ts[b] rearranged
to [N, T*D]:
    H   = [F|1]^T [F|1]                 (Gram + column sums + N)
    inv = (F^T F)^{-1}                  (Newton-Schulz, 5 iters, X0=(K/tr)I)
    M   = F^T Y, sumy = 1^T Y, sy2 = 1^T (Y*Y)
    q   = colsum(M * (inv M))           (= diag(M^T inv M))
    ss_res = sy2 - q                    (beta^T FtF beta ≈ beta^T M, err ~1e-12)
    ss_tot = sy2 - sumy^2/N + EPS
    r2  = 1 - ss_res/ss_tot ;  wsum_b = sum_td w[t,d] * r2[t,d]
    cov = FtF - s s^T / N ; c = 1/diag(cov) ; quad_b = c^T (cov*cov) c
loss = mean_b( -wsum_b/T ) + 0.1 * mean_b( quad_b - K )

The big tensor Y (12.3 MB/core) is streamed through SBUF once; all
reductions over N happen on the TensorEngine (PSUM accumulation over
47 chunks of 128 rows). sy2 needs Y^2, computed on ScalarE.
"""

import ml_dtypes
import numpy as np

import concourse.bacc as bacc
import concourse.bass as bass
import concourse.mybir as mybir
import concourse.tile as tile
from concourse.bass_utils import run_bass_kernel_spmd
from concourse.tile_rust import add_dep_helper

F32 = mybir.dt.float32
BF16 = mybir.dt.bfloat16
ALU = mybir.AluOpType
AX = mybir.AxisListType

B, T, N, K, D = 16, 32, 6000, 32, 8
NCORES = 8
JB = B // NCORES          # batch elements per core
NCH = 47                  # ceil(6000/128) chunks of 128 rows
NPAD = NCH * 128          # 6016
TD = T * D                # 256
FW = 34                   # per-chunk F block: 32 coeffs + ones col + pad
FROW = NCH * FW           # 1598
YROW = NCH * TD           # 12032
BLOCKS = (6, 6, 6, 6, 6, 6, 6, 5)  # chunk blocking of the Y stream
NS_ITERS = 4
EPS = 1e-8
DECAY = 0.9
PEN = 0.1

_CACHE = {}


def _build_program():
    nc = bacc.Bacc("TRN2", target_bir_lowering=False, debug=False)
    y_d = nc.declare_dram_parameter("y", [JB, 128, YROW], BF16, isOutput=False)
    f_d = nc.declare_dram_parameter("f", [JB, 128, FROW], BF16, isOutput=False)
    c_d = nc.declare_dram_parameter("c32", [32, 96], F32, isOutput=False)
    w_d = nc.declare_dram_parameter("w2", [1, TD], F32, isOutput=False)
    o_d = nc.declare_dram_parameter("out", [1, 2], F32, isOutput=True)

    with tile.TileContext(nc) as tc:
        with (
            tc.tile_pool(name="cpool", bufs=1) as cpool,
            tc.tile_pool(name="fpool", bufs=1) as fpool,
            tc.tile_pool(name="ypool", bufs=16) as ypool,
            tc.tile_pool(name="nsb", bufs=2) as nsb,
            tc.tile_pool(name="esb", bufs=2) as esb,
            # PSUM is 8 banks; every tag below occupies one bank.
            tc.tile_pool(name="ps", bufs=1, space="PSUM") as ps,
        ):
            # ---- PE warmup: junk matmuls fill the otherwise idle start
            # window so the HAM clock-gate reaches 2.4 GHz before the real
            # matmuls arrive (~3.4 us of sustained activity required).
            wtile = cpool.tile([128, 512], BF16)
            nc.gpsimd.memset(wtile, 0.01)
            wps = ps.tile([128, 512], F32, tag="wrm")
            for _ in range(14):
                nc.tensor.matmul(wps, wtile[:, 0:128], wtile,
                                 start=True, stop=True)

            # ---- DMAs: F first (needed by every matmul), then the Y
            # stream; triggers alternate between the two HWDGE issuing
            # engines (SP / ACT sequencer) so trigger issue is not serial
            # on one queue.
            ftile = fpool.tile([128, JB * FROW], BF16)
            fdma = nc.sync.dma_start(
                out=ftile[:, :].rearrange("p (j r) -> p j r", j=JB),
                in_=f_d[:, :, :].rearrange("j p r -> p j r"),
            )

            def fch(j, c):  # chunk-c F block [128, 33] (coeffs + ones)
                return ftile[:, j * FROW + c * FW: j * FROW + c * FW + 33]

            # ycomb tiles: [Y | Ysq] halves, one tile per (j, block).
            # The transfers are chained depth-2 (each trigger waits for the
            # completion two links back): at most two Y streams in flight,
            # so blocks arrive in order every ~2 us at full aggregate HBM
            # bandwidth instead of all 16 landing together at the end
            # (SDMA engines round-robin between all queues that have work).
            ycombs = {}
            ydmas = []
            dma_engines = [nc.sync, nc.scalar]
            for j in range(JB):
                c0 = 0
                for bi, blk in enumerate(BLOCKS):
                    yc = ypool.tile([128, blk * 512], BF16, tag="yc")
                    eng = dma_engines[(j * len(BLOCKS) + bi) % 2]
                    dma = eng.dma_start(
                        out=yc[:, 0:blk * TD],
                        in_=y_d[j, :, c0 * TD:(c0 + blk) * TD],
                    )
                    k = len(ydmas)
                    if k < 3:
                        add_dep_helper(dma.ins, fdma.ins, sync=True,
                                       reason="F streams solo first")
                    else:
                        add_dep_helper(dma.ins, ydmas[k - 3].ins, sync=True,
                                       reason="depth-3 Y stream chain")
                    ydmas.append(dma)
                    ycombs[(j, bi)] = yc
                    c0 += blk

            consts = cpool.tile([32, 96], F32)
            nc.gpsimd.dma_start(out=consts, in_=c_d[:, :])
            eye = consts[:, 0:32]
            twoI = consts[:, 32:64]
            ones2d = consts[:, 64:96]
            ones32 = consts[:, 64:65]

            w2sb = cpool.tile([33, TD], F32)
            nc.gpsimd.dma_start(out=w2sb[32:33, :], in_=w_d[:, :])

            # ---- Newton-Schulz inverse of FtF + correlation penalty, per j.
            # The Gram matrices are computed up front (dense PE work), but
            # the serial NS/corr chains (tiny matmul <-> DVE ping-pong)
            # would head-of-line-block the PE FIFO if emitted as one run.
            # Each PE step is wrapped in a closure and interleaved into the
            # streaming chunk loop below, so every step's DVE inputs are
            # long finished before the PE reaches its matmul.
            inv_sb = [None, None]
            quad_sb = [None, None]
            Hsb_j = [None, None]

            def emit_H(j):
                Hps = ps.tile([33, 33], F32, tag=f"H{j}")
                for c in range(NCH):
                    nc.tensor.matmul(
                        Hps, fch(j, c), fch(j, c),
                        start=(c == 0), stop=(c == NCH - 1),
                    )
                Hsb = nsb.tile([33, 33], F32, tag="Hsb")
                nc.vector.tensor_copy(Hsb, Hps)
                Hsb_j[j] = Hsb

            def make_steps(j):
                state = {}

                def s_trace():
                    Hsb = Hsb_j[j]
                    A = state["A"] = Hsb[0:32, 0:32]
                    state["s_row"] = Hsb[32:33, 0:32]
                    dm = nsb.tile([32, 32], F32, tag="dm")
                    nc.vector.tensor_mul(dm, A, eye)
                    dg = nsb.tile([32, 1], F32, tag="dg")
                    nc.vector.reduce_sum(dg, dm, axis=AX.X)
                    trp = ps.tile([32, 32], F32, tag="tns", bufs=2)
                    nc.tensor.matmul(trp[:, 0:1], ones2d, dg,
                                     start=True, stop=True)
                    rtr = nsb.tile([32, 1], F32, tag="rtr")
                    nc.vector.reciprocal(rtr, trp[:, 0:1])
                    c0v = nsb.tile([32, 1], F32, tag="c0v")
                    nc.vector.tensor_scalar_mul(c0v, rtr, float(K))
                    X = nsb.tile([32, 32], F32, tag="Xns", bufs=2 * NS_ITERS + 4)
                    nc.vector.tensor_scalar(X, eye, c0v, None, ALU.mult)
                    state["X"] = X
                steps = [s_trace]

                def ns_a():
                    t1 = ps.tile([32, 32], F32, tag="tns", bufs=2)
                    nc.tensor.matmul(t1, state["A"], state["X"],
                                     start=True, stop=True)
                    z = nsb.tile([32, 32], F32, tag="Zns",
                                 bufs=2 * NS_ITERS + 2)
                    nc.vector.tensor_sub(z, twoI, t1)
                    state["z"] = z

                def ns_b():
                    x2 = ps.tile([32, 32], F32, tag="tns", bufs=2)
                    nc.tensor.matmul(x2, state["X"], state["z"],
                                     start=True, stop=True)
                    Xn = nsb.tile([32, 32], F32, tag="Xns",
                                  bufs=2 * NS_ITERS + 4)
                    nc.vector.tensor_copy(Xn, x2)
                    state["X"] = Xn
                for _ in range(NS_ITERS):
                    steps += [ns_a, ns_b]

                def c_outer():
                    inv_sb[j] = state["X"]
                    outp = ps.tile([32, 32], F32, tag="tns", bufs=2)
                    nc.tensor.matmul(outp, state["s_row"], state["s_row"],
                                     start=True, stop=True)
                    covn = nsb.tile([32, 32], F32, tag="covn")
                    nc.vector.tensor_scalar_mul(covn, outp, 1.0 / N)
                    cov = nsb.tile([32, 32], F32, tag="cov")
                    nc.vector.tensor_sub(cov, state["A"], covn)
                    dm2 = nsb.tile([32, 32], F32, tag="dm2")
                    nc.vector.tensor_mul(dm2, cov, eye)
                    dg2 = nsb.tile([32, 1], F32, tag="dg2")
                    nc.vector.reduce_sum(dg2, dm2, axis=AX.X)
                    cv = nsb.tile([32, 1], F32, tag="cv")
                    nc.vector.reciprocal(cv, dg2)
                    A2 = nsb.tile([32, 32], F32, tag="A2")
                    nc.vector.tensor_mul(A2, cov, cov)
                    state["cv"] = cv
                    state["A2"] = A2

                def c_u():
                    ups = ps.tile([32, 32], F32, tag="tns", bufs=2)
                    nc.tensor.matmul(ups[:, 0:1], state["A2"], state["cv"],
                                     start=True, stop=True)
                    usb = nsb.tile([32, 1], F32, tag="usb")
                    nc.vector.tensor_copy(usb, ups[:, 0:1])
                    state["usb"] = usb

                def c_q():
                    qd = ps.tile([33, 32], F32, tag="tns", bufs=2)
                    nc.tensor.matmul(qd[32:33, 0:1], state["usb"], state["cv"],
                                     start=True, stop=True)
                    qsb = nsb.tile([33, 1], F32, tag="qsb")
                    nc.vector.tensor_copy(qsb[32:33, :], qd[32:33, 0:1])
                    quad_sb[j] = qsb
                steps += [c_outer, c_u, c_q]
                return steps

            emit_H(0)
            emit_H(1)
            pending = {0: make_steps(0), 1: make_steps(1)}

            # results staging: [wsum0, wsum1, quad0, quad1] (on partition 32,
            # where the GS row outputs live)
            wsout = cpool.tile([33, 4], F32)

            # ---- stream: square each block (alternating ScalarE / DVE),
            # then one matmul per chunk with rhs spanning [Y | Ysq]:
            #   GS[0:32, 0:256]   = F^T Y   (M)
            #   GS[32,   0:256]   = 1^T Y   (sumy)
            #   GS[32,   256:512] = 1^T Y^2 (sy2)
            for j in range(JB):
                GS = ps.tile([33, 512], F32, tag=f"GS{j}")
                steps = pending.pop(j)
                c0 = 0
                for bi, blk in enumerate(BLOCKS):
                    yc = ycombs[(j, bi)]
                    # squares run on ScalarE only: DVE stays responsive for
                    # the interleaved Newton-Schulz / epilogue chains
                    for p0 in range(0, blk, 6):
                        p1 = min(p0 + 6, blk)
                        nc.scalar.square(
                            yc[:, (blk + p0) * TD:(blk + p1) * TD],
                            yc[:, p0 * TD:p1 * TD],
                        )
                    rhs2 = yc[:, :].rearrange("p (two cd) -> p two cd", two=2)
                    for lc in range(blk):
                        c = c0 + lc
                        nc.tensor.matmul(
                            GS, fch(j, c),
                            rhs2[:, :, lc * TD:(lc + 1) * TD],
                            start=(c == 0), stop=(c == NCH - 1),
                        )
                        if c % 4 == 3 and steps:
                            steps.pop(0)()
                    c0 += blk
                while steps:
                    steps.pop(0)()

                # ---- per-j epilogue
                Gsb = esb.tile([33, 512], F32, tag="Gsb")
                nc.vector.tensor_copy(Gsb, GS)
                M = Gsb[0:32, 0:TD]
                sumy = Gsb[32:33, 0:TD]
                sy2row = Gsb[32:33, TD:2 * TD]

                Pps = ps.tile([32, TD], F32, tag="tPq")
                nc.tensor.matmul(Pps, inv_sb[j], M, start=True, stop=True)
                # ss_tot chain runs on DVE while PE computes P = inv M
                sumy2 = esb.tile([33, TD], F32, tag="sumy2")
                nc.vector.tensor_mul(sumy2[32:33, :], sumy, sumy)
                sstot_a = esb.tile([33, TD], F32, tag="sstot_a")
                nc.vector.tensor_scalar(
                    sstot_a[32:33, :], sumy2[32:33, :], -1.0 / N, EPS,
                    ALU.mult, ALU.add
                )
                sstot = esb.tile([33, TD], F32, tag="sstot")
                nc.vector.tensor_add(sstot[32:33, :], sstot_a[32:33, :], sy2row)
                rec = esb.tile([33, TD], F32, tag="rec")
                nc.vector.reciprocal(rec[32:33, :], sstot[32:33, :])
                W = esb.tile([32, TD], F32, tag="W")
                nc.vector.tensor_mul(W, M, Pps)
                qps = ps.tile([33, TD], F32, tag="tPq")
                nc.tensor.matmul(qps[32:33, :], ones32, W, start=True, stop=True)
                ssres = esb.tile([33, TD], F32, tag="ssres")
                nc.vector.tensor_sub(ssres[32:33, :], sy2row, qps[32:33, :])
                ratio = esb.tile([33, TD], F32, tag="ratio")
                nc.vector.tensor_mul(ratio[32:33, :], ssres[32:33, :],
                                     rec[32:33, :])
                r2 = esb.tile([33, TD], F32, tag="r2")
                nc.vector.tensor_scalar(r2[32:33, :], ratio[32:33, :],
                                        -1.0, 1.0, ALU.mult, ALU.add)
                scratch = esb.tile([33, TD], F32, tag="scratch")
                nc.vector.tensor_mul(scratch[32:33, :], r2[32:33, :],
                                     w2sb[32:33, :])
                nc.vector.reduce_sum(wsout[32:33, j:j + 1], scratch[32:33, :],
                                     axis=AX.X)
                nc.vector.tensor_copy(wsout[32:33, 2 + j:3 + j],
                                      quad_sb[j][32:33, :])

            outsb = cpool.tile([33, 2], F32)
            nc.vector.tensor_add(outsb[32:33, 0:1], wsout[32:33, 0:1],
                                 wsout[32:33, 1:2])
            nc.vector.tensor_add(outsb[32:33, 1:2], wsout[32:33, 2:3],
                                 wsout[32:33, 3:4])
            nc.gpsimd.dma_start(out=o_d[:, :], in_=outsb[32:33, :])

    nc.compile()
    return nc


def _prepare_in_maps(preds, y_ts, importance):
    preds = np.ascontiguousarray(preds, dtype=np.float32)
    y_ts = np.ascontiguousarray(y_ts, dtype=np.float32)
    importance = np.ascontiguousarray(importance, dtype=np.float32)

    bf16 = ml_dtypes.bfloat16

    # Y image: yimg[b, p, c*TD + t*D + d] = y_ts[b, t, c*128+p, d]
    ypad = np.zeros((B, T, NPAD, D), dtype=bf16)
    ypad[:, :, :N, :] = y_ts.astype(bf16)
    yimg = np.ascontiguousarray(
        ypad.reshape(B, T, NCH, 128, D).transpose(0, 3, 2, 1, 4)
    ).reshape(B, 128, YROW)

    # F image: fimg[b, p, c*FW + k] = preds[b, c*128+p, k]; col 32 = valid-mask
    fpad = np.zeros((B, NPAD, FW), dtype=bf16)
    fpad[:, :N, :K] = preds.astype(bf16)
    fpad[:, :N, K] = 1.0
    fimg = np.ascontiguousarray(
        fpad.reshape(B, NCH, 128, FW).transpose(0, 2, 1, 3)
    ).reshape(B, 128, FROW)

    c32 = np.zeros((32, 96), dtype=np.float32)
    c32[:, 0:32] = np.eye(32, dtype=np.float32)
    c32[:, 32:64] = 2.0 * np.eye(32, dtype=np.float32)
    c32[:, 64:96] = 1.0

    decay = DECAY ** np.arange(T, dtype=np.float32)
    w2 = (decay[:, None] * importance[None, :].astype(np.float32)).reshape(1, TD)
    w2 = np.ascontiguousarray(w2, dtype=np.float32)

    in_maps = []
    for i in range(NCORES):
        in_maps.append({
            "y": np.ascontiguousarray(yimg[i * JB:(i + 1) * JB]),
            "f": np.ascontiguousarray(fimg[i * JB:(i + 1) * JB]),
            "c32": c32,
            "w2": w2,
        })
    return in_maps


def _combine(results):
    loss = 0.0
    for r in results:
        w_total, q_total = float(r["out"][0, 0]), float(r["out"][0, 1])
        loss += (-w_total / T + PEN * (q_total - JB * K)) / B
    return np.float32(loss)


def run_on_device(preds, y_ts, importance, trace=False, **spmd_kwargs):
    if "nc" not in _CACHE:
        _CACHE["nc"] = _build_program()
    nc = _CACHE["nc"]
    in_maps = _prepare_in_maps(preds, y_ts, importance)
    res = run_bass_kernel_spmd(
        nc, in_maps, list(range(NCORES)), trace=trace, **spmd_kwargs
    )
    return _combine(res.results), res


def kernel(preds, y_ts, importance):
    loss, _ = run_on_device(preds, y_ts, importance, trace=False)
    return loss
